# revision 4
# baseline (speedup 1.0000x reference)
"""Causal self-attention Bass/Tile kernel for Trainium2, 8 NeuronCores.

Sharding: data-parallel over batch (2) x tensor-parallel over heads (16 -> 4
per core).  Core c handles batch b = c // 4 and heads 4*(c%4) .. 4*(c%4)+3.
Each core computes its heads' attention plus the partial output projection
(attn_out_c @ W_o[rows of its heads]); the host sums the 4 partials per batch.

Shapes (per core): x (2048, 1024) f32, W_q/W_k/W_v column slices (1024, 256),
W_o row slice (256, 1024), output partial (2048, 1024).

On-chip pipeline per core:
  A. load weights; load x tiles, transpose via PE (matmul w/ identity) -> xT
  B. QKV projections (contraction over e=1024 on partitions):
       QT/KT = (W^T x^T) in [d, s] layout (Q pre-scaled by 1/sqrt(64));
       V in [s, d] layout, cast to fp16, with a ones column appended per head
  C. per (head, q-block of 128):
       scores chunk = QT_blk^T @ KT chunk (PSUM, fp32, causal range only)
       mask-add on the diagonal 128x128 block (additive -1e30 upper triangle)
       row max (DVE reduce, negated) -> exp(s - m) via ACT -> P fp16 in SBUF
       P^T via DMA-transpose xbar (fp16) -> AV matmul with [V | 1] fp16
         -> PSUM [q, 64 + 1]; col 64 = softmax denominator l
       out_h = PSUM[:, :64] * (1/l)  (ACT copy with per-partition scale)
  D. transpose attn_out -> attn_outT, output projection, DMA partials out.
"""

import numpy as np

S = 2048
E = 1024
HPC = 4          # heads per core
D = 64
NCORES = 8
QB = S // 128    # 16 q-blocks
KC = 8           # e chunks of 128
SCALE = 0.125    # 1/sqrt(64)
NEG = -1.0e30

_CACHE = {}


def _build_nc():
    import concourse.bass as bass
    import concourse.mybir as mybir
    from concourse import tile

    f32 = mybir.dt.float32
    f16 = mybir.dt.float16
    X = mybir.AxisListType.X

    nc = bass.Bass()

    x_d = nc.declare_dram_parameter("x", [S, E], f32, isOutput=False)
    wq_d = nc.declare_dram_parameter("wq", [E, HPC * D], f32, isOutput=False)
    wk_d = nc.declare_dram_parameter("wk", [E, HPC * D], f32, isOutput=False)
    wv_d = nc.declare_dram_parameter("wv", [E, HPC * D], f32, isOutput=False)
    wo_d = nc.declare_dram_parameter("wo", [HPC * D, E], f32, isOutput=False)
    id_d = nc.declare_dram_parameter("ident", [128, 128], f32, isOutput=False)
    mask_d = nc.declare_dram_parameter("mask", [128, 128], f32, isOutput=False)
    out_d = nc.declare_dram_parameter("out", [S, E], f32, isOutput=True)

    with tile.TileContext(nc) as tc:
        with (
            tc.tile_pool(name="wpool", bufs=1) as wpool,
            tc.tile_pool(name="proj", bufs=1) as proj,
        ):
            # ---- persistent weight tiles ----
            wq = wpool.tile([128, KC, 256], f32)
            wk = wpool.tile([128, KC, 256], f32)
            wv = wpool.tile([128, KC, 256], f32)
            wo = wpool.tile([128, 2, E], f32)
            ident = wpool.tile([128, 128], f32)
            mask = wpool.tile([128, 128], f32)

            nc.sync.dma_start(wq[:], wq_d[:].rearrange("(c p) d -> p c d", p=128))
            nc.sync.dma_start(wk[:], wk_d[:].rearrange("(c p) d -> p c d", p=128))
            nc.sync.dma_start(wv[:], wv_d[:].rearrange("(c p) d -> p c d", p=128))
            nc.sync.dma_start(wo[:], wo_d[:].rearrange("(c p) e -> p c e", p=128))
            nc.sync.dma_start(ident[:], id_d[:])
            nc.sync.dma_start(mask[:], mask_d[:])

            # ---- projection outputs (persistent through phase C) ----
            qt = proj.tile([128, 2, S], f32)       # [d-pair, head-pair mt, s]
            kt = proj.tile([128, 2, S], f32)
            vones = proj.tile([128, QB, HPC, D + 1], f16)  # [keys, jblk, h, d|1]

            # ---- phase A: x load + transpose ----
            with tc.tile_pool(name="xtp", bufs=1) as xtp:
                xT = xtp.tile([128, KC, S], f32)   # [e%128, e//128, s]
                with (
                    tc.tile_pool(name="xin", bufs=3) as xin,
                    tc.tile_pool(name="tps", bufs=2, space="PSUM") as tps,
                ):
                    for i in range(QB):
                        xt = xin.tile([128, E], f32)
                        nc.sync.dma_start(xt[:], x_d[128 * i:128 * (i + 1), :])
                        for g in range(2):         # 2 groups of 4 e-chunks
                            tp = tps.tile([128, 512], f32)
                            for t in range(4):
                                c = 4 * g + t
                                nc.tensor.matmul(
                                    tp[:, 128 * t:128 * (t + 1)],
                                    xt[:, 128 * c:128 * (c + 1)],
                                    ident[:],
                                )
                            # scatter 4 transposed blocks to xT[:, c, s-block]
                            dst = xT[:, 4 * g:4 * g + 4, 128 * i:128 * (i + 1)]
                            if g == 0:
                                nc.vector.tensor_copy(dst, tp[:].rearrange("p (c s) -> p c s", c=4))
                            else:
                                nc.scalar.copy(dst, tp[:].rearrange("p (c s) -> p c s", c=4))

                # ---- phase B: QKV projections (uses xT) ----
                with tc.tile_pool(name="qkv", bufs=4, space="PSUM") as qkv:
                    # ones columns of vones
                    nc.vector.memset(vones[:, :, :, D:D + 1], 1.0)
                    # QT / KT: out [d 128-pair mt, s-chunk 512]
                    for mt in range(2):
                        for sc in range(4):
                            pq = qkv.tile([128, 512], f32, tag="qkv")
                            pk = qkv.tile([128, 512], f32, tag="qkv")
                            for c in range(KC):
                                nc.tensor.matmul(
                                    pq[:],
                                    wq[:, c, 128 * mt:128 * (mt + 1)],
                                    xT[:, c, 512 * sc:512 * (sc + 1)],
                                    start=(c == 0), stop=(c == KC - 1),
                                )
                            for c in range(KC):
                                nc.tensor.matmul(
                                    pk[:],
                                    wk[:, c, 128 * mt:128 * (mt + 1)],
                                    xT[:, c, 512 * sc:512 * (sc + 1)],
                                    start=(c == 0), stop=(c == KC - 1),
                                )
                            nc.scalar.mul(qt[:, mt, 512 * sc:512 * (sc + 1)], pq[:], SCALE)
                            nc.vector.tensor_copy(kt[:, mt, 512 * sc:512 * (sc + 1)], pk[:])
                    # V: out [s-block j, 4*64], cast fp16 into vones
                    for j in range(QB):
                        pv = qkv.tile([128, 256], f32, tag="qkv")
                        for c in range(KC):
                            nc.tensor.matmul(
                                pv[:],
                                xT[:, c, 128 * j:128 * (j + 1)],
                                wv[:, c, :],
                                start=(c == 0), stop=(c == KC - 1),
                            )
                        nc.vector.tensor_copy(
                            vones[:, j, :, 0:D],
                            pv[:].rearrange("p (h d) -> p h d", h=HPC),
                        )

            # ---- phase C: attention ----
            attn = proj.tile([128, QB, HPC * D], f32)   # [q, i, h*64]
            with (
                tc.tile_pool(name="sc", bufs=4, space="PSUM") as scp,
                tc.tile_pool(name="av", bufs=2, space="PSUM") as avp,
                tc.tile_pool(name="pbuf", bufs=2) as pbuf,
                tc.tile_pool(name="ptbuf", bufs=2) as ptbuf,
                tc.tile_pool(name="stat", bufs=4) as stat,
            ):
                for h in range(HPC):
                    mt, lo = h // 2, (h % 2) * 64
                    for i in range(QB):
                        kn = 128 * (i + 1)              # causal key range
                        nch = (kn + 511) // 512
                        qtb = qt[lo:lo + 64, mt, 128 * i:128 * (i + 1)]
                        chunks = []
                        for c in range(nch):
                            n = min(512, kn - 512 * c)
                            sp = scp.tile([128, 512], f32, tag="sc")
                            nc.tensor.matmul(
                                sp[:, :n], qtb,
                                kt[lo:lo + 64, mt, 512 * c:512 * c + n],
                            )
                            chunks.append((sp, n))
                        # causal mask on the diagonal 128 cols (last of range)
                        spl, nl = chunks[-1]
                        dslice = spl[:, nl - 128:nl]
                        nc.vector.tensor_add(dslice, dslice, mask[:])
                        # row max over all chunks -> -m
                        mparts = stat.tile([128, 4], f32, tag="mp")
                        for c, (sp, n) in enumerate(chunks):
                            nc.vector.reduce_max(mparts[:, c:c + 1], sp[:, :n], axis=X)
                        negm = stat.tile([128, 1], f32, tag="nm")
                        nc.vector.reduce_max(negm[:], mparts[:, :nch], axis=X, negate=True)
                        # exp -> P fp16
                        p = pbuf.tile([128, S], f16, tag="p")
                        for c, (sp, n) in enumerate(chunks):
                            nc.scalar.activation(
                                p[:, 512 * c:512 * c + n], sp[:, :n],
                                mybir.ActivationFunctionType.Exp,
                                bias=negm[:, 0:1], scale=1.0,
                            )
                        # P^T via DMA xbar, then AV (+ones) accumulation
                        pt = ptbuf.tile([128, QB, 128], f16, tag="pt")
                        for j in range(i + 1):
                            nc.sync.dma_start_transpose(
                                pt[:, j, :], p[:, 128 * j:128 * (j + 1)]
                            )
                        av = avp.tile([128, D + 1], f32, tag="av")
                        for j in range(i + 1):
                            nc.tensor.matmul(
                                av[:], pt[:, j, :], vones[:, j, h, :],
                                start=(j == 0), stop=(j == i),
                            )
                        rl = stat.tile([128, 1], f32, tag="rl")
                        nc.vector.reciprocal(rl[:], av[:, D:D + 1])
                        nc.scalar.mul(attn[:, i, D * h:D * (h + 1)], av[:, 0:D], rl[:, 0:1])

            # ---- phase D: attn_out^T + output projection ----
            with (
                tc.tile_pool(name="aot", bufs=1) as aotp,
                tc.tile_pool(name="tps2", bufs=2, space="PSUM") as tps2,
                tc.tile_pool(name="ops", bufs=2, space="PSUM") as ops,
                tc.tile_pool(name="osb", bufs=3) as osb,
            ):
                aot = aotp.tile([128, 2, S], f32)   # [d, db, s]
                for db in range(2):
                    for g in range(4):
                        tp = tps2.tile([128, 512], f32)
                        for t in range(4):
                            i = 4 * g + t
                            nc.tensor.matmul(
                                tp[:, 128 * t:128 * (t + 1)],
                                attn[:, i, 128 * db:128 * (db + 1)],
                                ident[:],
                            )
                        if g % 2 == 0:
                            nc.vector.tensor_copy(aot[:, db, 512 * g:512 * (g + 1)], tp[:])
                        else:
                            nc.scalar.copy(aot[:, db, 512 * g:512 * (g + 1)], tp[:])
                for sb in range(QB):
                    for ec in range(2):
                        po = ops.tile([128, 512], f32)
                        for kb in range(2):
                            nc.tensor.matmul(
                                po[:],
                                aot[:, kb, 128 * sb:128 * (sb + 1)],
                                wo[:, kb, 512 * ec:512 * (ec + 1)],
                                start=(kb == 0), stop=(kb == 1),
                            )
                        ob = osb.tile([128, 512], f32)
                        if ec == 0:
                            nc.scalar.copy(ob[:], po[:])
                        else:
                            nc.vector.tensor_copy(ob[:], po[:])
                        nc.sync.dma_start(
                            out_d[128 * sb:128 * (sb + 1), 512 * ec:512 * (ec + 1)],
                            ob[:],
                        )

    _split_excess_waits(nc)
    return nc


def _split_excess_waits(nc, maxw=1):
    """The walrus in this container only accepts one sync-wait per
    instruction; Tile's tail drain aggregates several.  Hoist excess waits
    onto preceding same-engine nops (in-order engines => equivalent)."""
    import concourse.mybir as mybir

    f = nc.m.functions[0]
    for b in f.blocks:
        insts = b.instructions
        i = 0
        while i < len(insts):
            inst = insts[i]
            si = inst.sync_info
            if si and si.on_wait and len(si.on_wait) > maxw:
                waits = list(si.on_wait)
                si.on_wait = waits[-maxw:]
                pos = i
                for w in waits[:-maxw]:
                    nop = nc.engines[inst.engine].nop(
                        nofuse=True, hint="wait_split"
                    ).ins
                    for bb in f.blocks:
                        L = bb.instructions
                        for k in range(len(L) - 1, -1, -1):
                            if L[k] is nop:
                                L.pop(k)
                                break
                    nsi = nop.sync_info
                    if nsi is None:
                        nop.sync_info = mybir.SyncInfo(on_wait=[w], on_update=[])
                    else:
                        nsi.on_wait = [w]
                    insts.insert(pos, nop)
                    pos += 1
                    i += 1
            i += 1


def _get_nc():
    if "nc" not in _CACHE:
        import concourse.mybir as mybir  # noqa: F401
        nc = _build_nc()
        _CACHE["nc"] = nc
    return _CACHE["nc"]


def _make_in_maps(x, W_q, W_k, W_v, W_o):
    ident = np.eye(128, dtype=np.float32)
    r = np.arange(128)
    mask = np.where(r[None, :] <= r[:, None], 0.0, NEG).astype(np.float32)
    in_maps = []
    for c in range(NCORES):
        b, g = c // 4, c % 4
        cs = slice(256 * g, 256 * (g + 1))
        in_maps.append({
            "x": np.ascontiguousarray(x[b]),
            "wq": np.ascontiguousarray(W_q[:, cs]),
            "wk": np.ascontiguousarray(W_k[:, cs]),
            "wv": np.ascontiguousarray(W_v[:, cs]),
            "wo": np.ascontiguousarray(W_o[cs, :]),
            "ident": ident,
            "mask": mask,
        })
    return in_maps


def run_on_hw(x, W_q, W_k, W_v, W_o, trace=False):
    from concourse.bass_utils import run_bass_kernel_spmd

    nc = _get_nc()
    in_maps = _make_in_maps(x, W_q, W_k, W_v, W_o)
    res = run_bass_kernel_spmd(nc, in_maps, core_ids=list(range(NCORES)),
                               trace=trace)
    parts = [res.results[c]["out"] for c in range(NCORES)]
    out = np.stack([
        parts[0] + parts[1] + parts[2] + parts[3],
        parts[4] + parts[5] + parts[6] + parts[7],
    ]).astype(np.float32)
    return out, res


def kernel(x, W_q, W_k, W_v, W_o):
    x = np.asarray(x, dtype=np.float32)
    W_q = np.asarray(W_q, dtype=np.float32)
    W_k = np.asarray(W_k, dtype=np.float32)
    W_v = np.asarray(W_v, dtype=np.float32)
    W_o = np.asarray(W_o, dtype=np.float32)
    out, _ = run_on_hw(x, W_q, W_k, W_v, W_o, trace=False)
    return out
